# revision 16
# baseline (speedup 1.0000x reference)
"""Baichuan attention layer (B=1, S=2048, E=4096, H=32, D=128) on 8 Trainium2
NeuronCores.

Sharding:
- QKV projection + RoPE + causal attention: tensor-parallel by head (4 heads
  per core). All per-head tensors live in transposed [feature, seq] layout so
  every matmul contracts over the partition dim with zero transposes:
    qkv^T[f, s]   = W @ X^T                (lhsT = W^T tiles, rhs = X^T tiles)
    scores^T[k,q] = K @ Q^T                (lhsT = K^T tile, rhs = Q^T block)
    att^T[d, q]   = V^T @ P^T              (lhsT = V tile,   rhs = exp tile)
  Softmax runs without max-subtraction (scores ~ N(0,1) after 1/sqrt(D), fp32
  exp is safe); the denominator is accumulated with an all-ones [128,128] lhsT
  matmul so the k-sum lands in PSUM already replicated across partitions.
- One AllGather of att^T [512, 2048] bf16 per core -> full att^T [4096, 2048].
- o_proj: column-parallel (each core computes its 512 output columns for the
  full sequence, using its slice of w_o). Host concatenates along E.

All matmuls in bf16 with fp32 PSUM accumulation.
"""

import importlib.util
import sys
import types

import numpy as np
import ml_dtypes

BF16NP = ml_dtypes.bfloat16

B, S, E = 1, 2048, 4096
H, D = 32, 128
NCORES = 8
HPC = H // NCORES          # heads per core = 4
P = 128                    # partitions
SBLK = 512                 # seq block (matmul free dim)
NSBLK = S // SBLK          # 4
ET = E // P                # 32 e-tiles
NF = 3 * HPC               # 12 f-tiles per core (q0..3, k4..7, v8..11)
KT = S // P                # 16 k-tiles
ECOLS = E // NCORES        # 512 output columns per core
SCALE = 1.0 / float(np.sqrt(D))


def _install_ntff_hook():
    """antenv.axon_hooks is absent in this image; recreate it from trn_boot's
    ctypes shim so run_bass_kernel_spmd(trace=True) can capture NTFF traces."""
    if "antenv.axon_hooks" in sys.modules:
        return
    try:
        spec = importlib.util.spec_from_file_location(
            "trn_boot", "/root/.axon_site/trn_agent_boot/trn_boot.py")
        tb = importlib.util.module_from_spec(spec)
        spec.loader.exec_module(tb)
        hook = tb._ntff_profile_via_ctypes("/opt/axon/libaxon_pjrt.so")
    except Exception:
        hook = None
    mod = types.ModuleType("antenv.axon_hooks")
    mod.get_axon_ntff_profile_hook = lambda: hook
    mod.set_axon_ntff_profile_hook = lambda h: None
    sys.modules["antenv.axon_hooks"] = mod


_install_ntff_hook()

import concourse.bass as bass  # noqa: E402
import concourse.mybir as mybir  # noqa: E402
import concourse.tile as tile  # noqa: E402
from concourse import bacc  # noqa: E402
from concourse.bass import ts  # noqa: E402
from concourse.bass_utils import run_bass_kernel_spmd  # noqa: E402


def _maybe_patch_ldw_opt():
    """Optional experiment: walrus runs with --enable-ldw-opt=false; flipping
    it lets codegen pipeline LDWEIGHTS with matmuls.  Gated on BASS_LDW_OPT=1
    until proven correct."""
    import os
    if os.environ.get("BASS_LDW_OPT") != "1":
        return
    from concourse import bass_utils as bu
    if getattr(bu, "_ldw_patched", False):
        return
    orig_run = bu.run_command

    def patched(cmd, *a, **kw):
        if isinstance(cmd, list):
            cmd = [c.replace("--enable-ldw-opt=false", "--enable-ldw-opt=true")
                   if isinstance(c, str) else c for c in cmd]
        return orig_run(cmd, *a, **kw)

    bu.run_command = patched
    bu._ldw_patched = True


_maybe_patch_ldw_opt()

BF16 = mybir.dt.bfloat16
F32 = mybir.dt.float32

_NC_CACHE = None


def build():
    global _NC_CACHE
    if _NC_CACHE is not None:
        return _NC_CACHE
    nc = bacc.Bacc("TRN2", target_bir_lowering=False, debug=False,
                   num_devices=NCORES)

    xt_ext = nc.dram_tensor("xt", [E, S], BF16, kind="ExternalInput")
    wt_ext = nc.dram_tensor("wt", [E, NF * P], BF16, kind="ExternalInput")
    wot_ext = nc.dram_tensor("wot", [E, ECOLS], BF16, kind="ExternalInput")
    cost_ext = nc.dram_tensor("cost", [D, S], F32, kind="ExternalInput")
    sint_ext = nc.dram_tensor("sint", [D, S], F32, kind="ExternalInput")
    rt_ext = nc.dram_tensor("rt", [D, D], BF16, kind="ExternalInput")
    masks_ext = nc.dram_tensor("masks", [4, P, SBLK], BF16, kind="ExternalInput")
    ones_ext = nc.dram_tensor("ones", [P, P], BF16, kind="ExternalInput")
    ident_ext = nc.dram_tensor("ident", [P, P], BF16, kind="ExternalInput")
    out_ext = nc.dram_tensor("out", [S, ECOLS], F32, kind="ExternalOutput")

    # Two AllGathers (heads 0-1, heads 2-3) so the first overlaps the second
    # half of attention.  ccoutA rank-r block = rows [256r, 256r+256) =
    # global heads {4r, 4r+1}; ccoutB likewise heads {4r+2, 4r+3}.
    ccinA = nc.dram_tensor("ccinA", [2 * P, S], BF16)
    ccinB = nc.dram_tensor("ccinB", [2 * P, S], BF16)
    ccoutA = nc.dram_tensor("ccoutA", [NCORES * 2 * P, S], BF16,
                            addr_space="Shared")
    ccoutB = nc.dram_tensor("ccoutB", [NCORES * 2 * P, S], BF16,
                            addr_space="Shared")

    xt_t = xt_ext.ap().rearrange("(eo p) s -> p eo s", p=P)
    wt_t = wt_ext.ap().rearrange("(eo p) f -> p eo f", p=P)
    wot_t = wot_ext.ap().rearrange("(fo p) e -> p fo e", p=P)
    masks_t = masks_ext.ap().rearrange("r p q -> p r q")
    ccinA_t = ccinA.ap().rearrange("(h p) s -> p h s", p=P)
    ccinB_t = ccinB.ap().rearrange("(h p) s -> p h s", p=P)
    # [p, (c,h), s] with f-tile index (c, h) -> global head 4c + h(+2 for B)
    ccoutA_t = ccoutA.ap().rearrange("(c h p) s -> p (c h) s", p=P, h=2)
    ccoutB_t = ccoutB.ap().rearrange("(c h p) s -> p (c h) s", p=P, h=2)

    with tile.TileContext(nc) as tc:
        with (
            tc.tile_pool(name="cst", bufs=1) as cst,
            tc.tile_pool(name="ropeT", bufs=1) as ropeT_pool,
            tc.tile_pool(name="vT", bufs=1) as vT_pool,
        ):
            rt_sb = cst.tile([D, D], BF16)
            nc.sync.dma_start(rt_sb[:], rt_ext.ap())
            ones_sb = cst.tile([P, P], BF16)
            nc.sync.dma_start(ones_sb[:], ones_ext.ap())
            ident_sb = cst.tile([P, P], BF16)
            nc.sync.dma_start(ident_sb[:], ident_ext.ap())
            masks_sb = cst.tile([P, 4, SBLK], BF16)
            nc.sync.dma_start(masks_sb[:], masks_t)

            # q^T and k^T after RoPE: [128, 8, 2048]; v^T raw: [128, 4, 2048]
            ropeT_sb = ropeT_pool.tile([P, 2 * HPC, S], BF16)
            vT_sb = vT_pool.tile([P, HPC, S], BF16)

            # ---------------- Phase 1: QKV projection + RoPE -------------
            with (
                tc.tile_pool(name="xt", bufs=2) as xt_pool,
                tc.tile_pool(name="wq", bufs=3) as w_pool,
                tc.tile_pool(name="cs", bufs=2) as cs_pool,
                tc.tile_pool(name="qkc", bufs=3) as qkc_pool,
                tc.tile_pool(name="rtmp", bufs=4) as rtmp_pool,
                tc.tile_pool(name="ps_qkv", bufs=3, space="PSUM") as ps_qkv,
                tc.tile_pool(name="ps_rot", bufs=2, space="PSUM") as ps_rot,
            ):
                for b in range(NSBLK):
                    sblk = ts(b, SBLK)
                    xt_sb = xt_pool.tile([P, ET, SBLK], BF16, tag="xt")
                    # chunked so the first matmuls start before the whole
                    # block has landed (and chunks spread across DMA queues)
                    for ch in range(4):
                        nc.sync.dma_start(xt_sb[:, ts(ch, ET // 4), :],
                                          xt_t[:, ts(ch, ET // 4), sblk])
                    cos_sb = cs_pool.tile([D, SBLK], F32, tag="cos")
                    nc.sync.dma_start(cos_sb[:], cost_ext.ap()[:, sblk])
                    sin_sb = cs_pool.tile([D, SBLK], F32, tag="sin")
                    nc.sync.dma_start(sin_sb[:], sint_ext.ap()[:, sblk])

                    for f in range(NF):
                        w_sb = w_pool.tile([P, ET, P], BF16, tag="w")
                        nc.sync.dma_start(w_sb[:], wt_t[:, :, ts(f, P)])
                        acc_ps = ps_qkv.tile([P, SBLK], F32, tag="qkv")
                        for e in range(ET):
                            nc.tensor.matmul(
                                acc_ps[:], w_sb[:, e, :], xt_sb[:, e, :],
                                start=(e == 0), stop=(e == ET - 1),
                            )
                        if f < 2 * HPC:
                            # q/k: RoPE.  bf16 copy for the rotate matmul
                            qk_sb = qkc_pool.tile([P, SBLK], BF16, tag="qkc")
                            nc.any.tensor_copy(qk_sb[:], acc_ps[:])
                            rot_ps = ps_rot.tile([P, SBLK], F32, tag="rot")
                            nc.tensor.matmul(rot_ps[:], rt_sb[:], qk_sb[:],
                                             start=True, stop=True)
                            t1 = rtmp_pool.tile([P, SBLK], F32, tag="t1")
                            nc.vector.tensor_mul(out=t1[:], in0=acc_ps[:],
                                                 in1=cos_sb[:])
                            t2 = rtmp_pool.tile([P, SBLK], F32, tag="t2")
                            nc.vector.tensor_mul(out=t2[:], in0=rot_ps[:],
                                                 in1=sin_sb[:])
                            nc.vector.tensor_add(
                                out=ropeT_sb[:, f, sblk], in0=t1[:], in1=t2[:])
                        else:
                            # v: plain copy to bf16
                            nc.any.tensor_copy(vT_sb[:, f - 2 * HPC, sblk],
                                               acc_ps[:])

            # ------------- Phase 2: attention per head + AllGather -------
            with (
                tc.tile_pool(name="wot", bufs=1) as wot_pool,
                tc.tile_pool(name="attnT", bufs=1) as attnT_pool,
            ):
                wot_sb = wot_pool.tile([P, ET, ECOLS], BF16)
                nc.sync.dma_start(wot_sb[:], wot_t)
                attnT_sb = attnT_pool.tile([P, HPC, S], BF16)

                with tc.tile_pool(name="vsb", bufs=1) as vsb_pool:
                    # All V transposes up front (own PSUM scope):
                    # v^T [d,s] -> V [s,d] per 128x128 tile
                    v_all_sb = vsb_pool.tile([P, HPC * KT, P], BF16)
                    with tc.tile_pool(name="ps_vtr", bufs=4,
                                      space="PSUM") as ps_vtr:
                        for h in range(HPC):
                            for kt in range(KT):
                                vt_ps = ps_vtr.tile([P, P], BF16, tag="vtr")
                                nc.tensor.transpose(vt_ps[:],
                                                    vT_sb[:, h, ts(kt, P)],
                                                    ident_sb[:])
                                nc.any.tensor_copy(
                                    v_all_sb[:, h * KT + kt, :], vt_ps[:])

                    with (
                        tc.tile_pool(name="exp", bufs=6) as exp_pool,
                        tc.tile_pool(name="rcp", bufs=2) as rcp_pool,
                        tc.tile_pool(name="dn", bufs=2) as dn_pool,
                        tc.tile_pool(name="ps_sc", bufs=4, space="PSUM") as ps_sc,
                        tc.tile_pool(name="ps_av", bufs=2, space="PSUM") as ps_av,
                        tc.tile_pool(name="ps_den", bufs=2, space="PSUM") as ps_den,
                    ):
                        for h in range(HPC):
                            v_sb = v_all_sb[:, h * KT:(h + 1) * KT, :]
                            qh = ropeT_sb[:, h, :]
                            kh = ropeT_sb[:, HPC + h, :]
                            for j in range(NSBLK):
                                nkt = 4 * j + 4
                                av_ps = ps_av.tile([P, SBLK], F32, tag="av")
                                den_sb = dn_pool.tile([P, SBLK], F32, tag="dn")
                                for i in range(nkt):
                                    sc_ps = ps_sc.tile([P, SBLK], F32, tag="sc")
                                    nc.tensor.matmul(sc_ps[:], kh[:, ts(i, P)],
                                                     qh[:, ts(j, SBLK)],
                                                     start=True, stop=True)
                                    exp_sb = exp_pool.tile([P, SBLK], BF16,
                                                           tag="exp")
                                    nc.scalar.activation(
                                        exp_sb[:], sc_ps[:],
                                        mybir.ActivationFunctionType.Exp,
                                        scale=SCALE)
                                    if i >= 4 * j:
                                        nc.vector.tensor_mul(
                                            out=exp_sb[:], in0=exp_sb[:],
                                            in1=masks_sb[:, i - 4 * j, :])
                                    # k-tile part of the denominator on DVE
                                    # (keeps PE free): per-partition partials
                                    # in f32, partition-reduced below
                                    if i == 0:
                                        nc.vector.tensor_copy(den_sb[:],
                                                              exp_sb[:])
                                    else:
                                        nc.vector.tensor_add(
                                            out=den_sb[:], in0=den_sb[:],
                                            in1=exp_sb[:])
                                    nc.tensor.matmul(
                                        av_ps[:], v_sb[:, i, :], exp_sb[:],
                                        start=(i == 0), stop=(i == nkt - 1))
                                # partition reduction: ones.T @ den -> [128,q]
                                # (all-ones lhsT replicates the k-sum)
                                den_bf = dn_pool.tile([P, SBLK], BF16,
                                                      tag="dnb")
                                nc.vector.tensor_copy(den_bf[:], den_sb[:])
                                den_ps = ps_den.tile([P, SBLK], F32, tag="den")
                                nc.tensor.matmul(den_ps[:], ones_sb[:],
                                                 den_bf[:],
                                                 start=True, stop=True)
                                recip_sb = rcp_pool.tile([P, SBLK], F32,
                                                         tag="rcp")
                                nc.vector.reciprocal(recip_sb[:], den_ps[:])
                                nc.vector.tensor_mul(
                                    out=attnT_sb[:, h, ts(j, SBLK)],
                                    in0=av_ps[:], in1=recip_sb[:])

                            if h == 1:
                                nc.sync.dma_start(ccinA_t, attnT_sb[:, 0:2, :])
                                nc.gpsimd.collective_compute(
                                    "AllGather", mybir.AluOpType.bypass,
                                    replica_groups=[list(range(NCORES))],
                                    ins=[ccinA.ap()], outs=[ccoutA.ap()],
                                )
                            elif h == 3:
                                nc.sync.dma_start(ccinB_t, attnT_sb[:, 2:4, :])
                                nc.gpsimd.collective_compute(
                                    "AllGather", mybir.AluOpType.bypass,
                                    replica_groups=[list(range(NCORES))],
                                    ins=[ccinB.ap()], outs=[ccoutB.ap()],
                                )

                # ---------------- Phase 3: o_proj ------------------------
                with (
                    tc.tile_pool(name="at", bufs=6) as at_pool,
                    tc.tile_pool(name="osb", bufs=3) as osb_pool,
                    tc.tile_pool(name="ps_out", bufs=6, space="PSUM") as ps_out,
                ):
                    # f-tile (c, hh) in ccoutA/B block c -> global head 4c+hh
                    # (A: hh in {0,1}; B: hh in {2,3}).  A-tiles first so the
                    # first half of each accumulation overlaps AllGather B.
                    for st in range(S // P):
                        a_sbA = at_pool.tile([P, 16, P], BF16, tag="atA")
                        nc.sync.dma_start(a_sbA[:], ccoutA_t[:, :, ts(st, P)])
                        a_sbB = at_pool.tile([P, 16, P], BF16, tag="atB")
                        nc.sync.dma_start(a_sbB[:], ccoutB_t[:, :, ts(st, P)])
                        o_ps = ps_out.tile([P, ECOLS], F32, tag="out")
                        for idx in range(16):
                            g = 4 * (idx // 2) + (idx % 2)
                            nc.tensor.matmul(o_ps[:], a_sbA[:, idx, :],
                                             wot_sb[:, g, :],
                                             start=(idx == 0), stop=False)
                        for idx in range(16):
                            g = 4 * (idx // 2) + 2 + (idx % 2)
                            nc.tensor.matmul(o_ps[:], a_sbB[:, idx, :],
                                             wot_sb[:, g, :],
                                             start=False, stop=(idx == 15))
                        o_sb = osb_pool.tile([P, ECOLS], F32, tag="osb")
                        nc.any.tensor_copy(o_sb[:], o_ps[:])
                        nc.sync.dma_start(out_ext.ap()[ts(st, P), :], o_sb[:])

    nc.compile()
    _NC_CACHE = nc
    return nc


def _prep_inputs(hidden_states, cos, sin, w_pack, w_o):
    hs = np.asarray(hidden_states, dtype=np.float32).reshape(S, E)
    xt = np.ascontiguousarray(hs.T).astype(BF16NP)
    cost = np.ascontiguousarray(np.asarray(cos, dtype=np.float32).T)
    sint = np.ascontiguousarray(np.asarray(sin, dtype=np.float32).T)
    w_pack = np.asarray(w_pack, dtype=np.float32)
    w_o = np.asarray(w_o, dtype=np.float32)

    # rotate-half as a matmul: rot = R @ q  (R[d'<64, d'+64] = -1; R[d'>=64, d'-64] = +1)
    R = np.zeros((D, D), dtype=np.float32)
    half = D // 2
    for dp in range(half):
        R[dp, dp + half] = -1.0
    for dp in range(half, D):
        R[dp, dp - half] = 1.0
    rt = np.ascontiguousarray(R.T).astype(BF16NP)

    masks = np.zeros((4, P, SBLK), dtype=np.float32)
    kk = np.arange(P)[:, None]
    qq = np.arange(SBLK)[None, :]
    for r in range(4):
        masks[r] = (P * r + kk <= qq).astype(np.float32)
    masks = masks.astype(BF16NP)

    ones = np.ones((P, P), dtype=BF16NP)
    ident = np.eye(P, dtype=np.float32).astype(BF16NP)

    in_maps = []
    hw = E // NCORES  # 512 head-rows per core in each of q/k/v
    for c in range(NCORES):
        rows = slice(c * hw, (c + 1) * hw)
        wqkv = np.concatenate(
            [w_pack[rows], w_pack[E:][rows], w_pack[2 * E:][rows]], axis=0)
        wt = np.ascontiguousarray(wqkv.T).astype(BF16NP)
        wot = np.ascontiguousarray(w_o[rows].T).astype(BF16NP)
        in_maps.append({
            "xt": xt, "wt": wt, "wot": wot,
            "cost": cost, "sint": sint,
            "rt": rt, "masks": masks, "ones": ones, "ident": ident,
        })
    return in_maps


def run(trace=False, trace_cores=None, **inputs):
    nc = build()
    in_maps = _prep_inputs(**inputs)
    res = run_bass_kernel_spmd(
        nc, in_maps, core_ids=list(range(NCORES)),
        trace=trace, trace_cores=trace_cores,
    )
    out = np.concatenate([res.results[c]["out"] for c in range(NCORES)], axis=1)
    return out.reshape(B, S, E).astype(np.float32), res


def kernel(**inputs) -> np.ndarray:
    out, _ = run(trace=False, **inputs)
    return out


# revision 19
# speedup vs baseline: 1.0670x; 1.0670x over previous
"""Baichuan attention layer (B=1, S=2048, E=4096, H=32, D=128) on 8 Trainium2
NeuronCores.

Sharding:
- QKV projection + RoPE + causal attention: tensor-parallel by head (4 heads
  per core). All per-head tensors live in transposed [feature, seq] layout so
  every matmul contracts over the partition dim with zero transposes:
    qkv^T[f, s]   = W @ X^T                (lhsT = W^T tiles, rhs = X^T tiles)
    scores^T[k,q] = K @ Q^T                (lhsT = K^T tile, rhs = Q^T block)
    att^T[d, q]   = V^T @ P^T              (lhsT = V tile,   rhs = exp tile)
  Softmax runs without max-subtraction (scores ~ N(0,1) after 1/sqrt(D), fp32
  exp is safe); the denominator is accumulated with an all-ones [128,128] lhsT
  matmul so the k-sum lands in PSUM already replicated across partitions.
- One AllGather of att^T [512, 2048] bf16 per core -> full att^T [4096, 2048].
- o_proj: column-parallel (each core computes its 512 output columns for the
  full sequence, using its slice of w_o). Host concatenates along E.

All matmuls in bf16 with fp32 PSUM accumulation.
"""

import importlib.util
import sys
import types

import numpy as np
import ml_dtypes

BF16NP = ml_dtypes.bfloat16

B, S, E = 1, 2048, 4096
H, D = 32, 128
NCORES = 8
HPC = H // NCORES          # heads per core = 4
P = 128                    # partitions
SBLK = 512                 # seq block (matmul free dim)
NSBLK = S // SBLK          # 4
ET = E // P                # 32 e-tiles
NF = 3 * HPC               # 12 f-tiles per core (q0..3, k4..7, v8..11)
KT = S // P                # 16 k-tiles
ECOLS = E // NCORES        # 512 output columns per core
SCALE = 1.0 / float(np.sqrt(D))


def _install_ntff_hook():
    """antenv.axon_hooks is absent in this image; recreate it from trn_boot's
    ctypes shim so run_bass_kernel_spmd(trace=True) can capture NTFF traces."""
    if "antenv.axon_hooks" in sys.modules:
        return
    try:
        spec = importlib.util.spec_from_file_location(
            "trn_boot", "/root/.axon_site/trn_agent_boot/trn_boot.py")
        tb = importlib.util.module_from_spec(spec)
        spec.loader.exec_module(tb)
        hook = tb._ntff_profile_via_ctypes("/opt/axon/libaxon_pjrt.so")
    except Exception:
        hook = None
    mod = types.ModuleType("antenv.axon_hooks")
    mod.get_axon_ntff_profile_hook = lambda: hook
    mod.set_axon_ntff_profile_hook = lambda h: None
    sys.modules["antenv.axon_hooks"] = mod


_install_ntff_hook()

import concourse.bass as bass  # noqa: E402
import concourse.mybir as mybir  # noqa: E402
import concourse.tile as tile  # noqa: E402
from concourse import bacc  # noqa: E402
from concourse.bass import ts  # noqa: E402
from concourse.bass_utils import run_bass_kernel_spmd  # noqa: E402


def _maybe_patch_ldw_opt():
    """Optional experiment: walrus runs with --enable-ldw-opt=false; flipping
    it lets codegen pipeline LDWEIGHTS with matmuls.  Gated on BASS_LDW_OPT=1
    until proven correct."""
    import os
    if os.environ.get("BASS_LDW_OPT") != "1":
        return
    from concourse import bass_utils as bu
    if getattr(bu, "_ldw_patched", False):
        return
    orig_run = bu.run_command

    def patched(cmd, *a, **kw):
        if isinstance(cmd, list):
            cmd = [c.replace("--enable-ldw-opt=false", "--enable-ldw-opt=true")
                   if isinstance(c, str) else c for c in cmd]
        return orig_run(cmd, *a, **kw)

    bu.run_command = patched
    bu._ldw_patched = True


_maybe_patch_ldw_opt()

BF16 = mybir.dt.bfloat16
F32 = mybir.dt.float32

_NC_CACHE = None


def build():
    global _NC_CACHE
    if _NC_CACHE is not None:
        return _NC_CACHE
    nc = bacc.Bacc("TRN2", target_bir_lowering=False, debug=False,
                   num_devices=NCORES)

    xt_ext = nc.dram_tensor("xt", [E, S], BF16, kind="ExternalInput")
    wt_ext = nc.dram_tensor("wt", [E, NF * P], BF16, kind="ExternalInput")
    wot_ext = nc.dram_tensor("wot", [E, ECOLS], BF16, kind="ExternalInput")
    cost_ext = nc.dram_tensor("cost", [D, S], F32, kind="ExternalInput")
    sint_ext = nc.dram_tensor("sint", [D, S], F32, kind="ExternalInput")
    rt_ext = nc.dram_tensor("rt", [D, D], BF16, kind="ExternalInput")
    masks_ext = nc.dram_tensor("masks", [4, P, SBLK], BF16, kind="ExternalInput")
    ones_ext = nc.dram_tensor("ones", [P, P], BF16, kind="ExternalInput")
    ident_ext = nc.dram_tensor("ident", [P, P], BF16, kind="ExternalInput")
    out_ext = nc.dram_tensor("out", [S, ECOLS], F32, kind="ExternalOutput")

    # Two AllGathers (heads 0-1, heads 2-3) so the first overlaps the second
    # half of attention.  ccoutA rank-r block = rows [256r, 256r+256) =
    # global heads {4r, 4r+1}; ccoutB likewise heads {4r+2, 4r+3}.
    ccinA = nc.dram_tensor("ccinA", [2 * P, S], BF16)
    ccinB = nc.dram_tensor("ccinB", [2 * P, S], BF16)
    ccoutA = nc.dram_tensor("ccoutA", [NCORES * 2 * P, S], BF16,
                            addr_space="Shared")
    ccoutB = nc.dram_tensor("ccoutB", [NCORES * 2 * P, S], BF16,
                            addr_space="Shared")

    xt_t = xt_ext.ap().rearrange("(eo p) s -> p eo s", p=P)
    wt_t = wt_ext.ap().rearrange("(eo p) f -> p eo f", p=P)
    wot_t = wot_ext.ap().rearrange("(fo p) e -> p fo e", p=P)
    masks_t = masks_ext.ap().rearrange("r p q -> p r q")
    ccinA_t = ccinA.ap().rearrange("(h p) s -> p h s", p=P)
    ccinB_t = ccinB.ap().rearrange("(h p) s -> p h s", p=P)
    # [p, (c,h), s] with f-tile index (c, h) -> global head 4c + h(+2 for B)
    ccoutA_t = ccoutA.ap().rearrange("(c h p) s -> p (c h) s", p=P, h=2)
    ccoutB_t = ccoutB.ap().rearrange("(c h p) s -> p (c h) s", p=P, h=2)

    with tile.TileContext(nc) as tc:
        with (
            tc.tile_pool(name="cst", bufs=1) as cst,
            tc.tile_pool(name="ropeT", bufs=1) as ropeT_pool,
            tc.tile_pool(name="vT", bufs=1) as vT_pool,
        ):
            rt_sb = cst.tile([D, D], BF16)
            nc.sync.dma_start(rt_sb[:], rt_ext.ap())
            ones_sb = cst.tile([P, P], BF16)
            nc.sync.dma_start(ones_sb[:], ones_ext.ap())
            ident_sb = cst.tile([P, P], BF16)
            nc.sync.dma_start(ident_sb[:], ident_ext.ap())
            masks_sb = cst.tile([P, 4, SBLK], BF16)
            nc.sync.dma_start(masks_sb[:], masks_t)

            # q^T and k^T after RoPE: [128, 8, 2048]; v^T raw: [128, 4, 2048]
            ropeT_sb = ropeT_pool.tile([P, 2 * HPC, S], BF16)
            vT_sb = vT_pool.tile([P, HPC, S], BF16)

            # ---------------- Phase 1: QKV projection + RoPE -------------
            with (
                tc.tile_pool(name="xt", bufs=2) as xt_pool,
                tc.tile_pool(name="wq", bufs=3) as w_pool,
                tc.tile_pool(name="cs", bufs=2) as cs_pool,
                tc.tile_pool(name="qkc", bufs=3) as qkc_pool,
                tc.tile_pool(name="rtmp", bufs=4) as rtmp_pool,
                tc.tile_pool(name="ps_qkv", bufs=3, space="PSUM") as ps_qkv,
                tc.tile_pool(name="ps_rot", bufs=2, space="PSUM") as ps_rot,
            ):
                for b in range(NSBLK):
                    sblk = ts(b, SBLK)
                    xt_sb = xt_pool.tile([P, ET, SBLK], BF16, tag="xt")
                    # chunked so the first matmuls start before the whole
                    # block has landed (and chunks spread across DMA queues)
                    for ch in range(4):
                        nc.sync.dma_start(xt_sb[:, ts(ch, ET // 4), :],
                                          xt_t[:, ts(ch, ET // 4), sblk])
                    cos_sb = cs_pool.tile([D, SBLK], F32, tag="cos")
                    nc.sync.dma_start(cos_sb[:], cost_ext.ap()[:, sblk])
                    sin_sb = cs_pool.tile([D, SBLK], F32, tag="sin")
                    nc.sync.dma_start(sin_sb[:], sint_ext.ap()[:, sblk])

                    for f in range(NF):
                        w_sb = w_pool.tile([P, ET, P], BF16, tag="w")
                        nc.sync.dma_start(w_sb[:], wt_t[:, :, ts(f, P)])
                        acc_ps = ps_qkv.tile([P, SBLK], F32, tag="qkv")
                        for e in range(ET):
                            nc.tensor.matmul(
                                acc_ps[:], w_sb[:, e, :], xt_sb[:, e, :],
                                start=(e == 0), stop=(e == ET - 1),
                            )
                        if f < 2 * HPC:
                            # q/k: RoPE.  bf16 copy for the rotate matmul
                            qk_sb = qkc_pool.tile([P, SBLK], BF16, tag="qkc")
                            nc.any.tensor_copy(qk_sb[:], acc_ps[:])
                            rot_ps = ps_rot.tile([P, SBLK], F32, tag="rot")
                            nc.tensor.matmul(rot_ps[:], rt_sb[:], qk_sb[:],
                                             start=True, stop=True)
                            t1 = rtmp_pool.tile([P, SBLK], F32, tag="t1")
                            nc.vector.tensor_mul(out=t1[:], in0=acc_ps[:],
                                                 in1=cos_sb[:])
                            t2 = rtmp_pool.tile([P, SBLK], F32, tag="t2")
                            nc.vector.tensor_mul(out=t2[:], in0=rot_ps[:],
                                                 in1=sin_sb[:])
                            nc.vector.tensor_add(
                                out=ropeT_sb[:, f, sblk], in0=t1[:], in1=t2[:])
                        else:
                            # v: plain copy to bf16
                            nc.any.tensor_copy(vT_sb[:, f - 2 * HPC, sblk],
                                               acc_ps[:])

            # ------------- Phase 2: attention per head + AllGather -------
            with (
                tc.tile_pool(name="wot", bufs=1) as wot_pool,
                tc.tile_pool(name="attnT", bufs=1) as attnT_pool,
            ):
                wot_sb = wot_pool.tile([P, ET, ECOLS], BF16)
                nc.sync.dma_start(wot_sb[:], wot_t)
                attnT_sb = attnT_pool.tile([P, HPC, S], BF16)

                with tc.tile_pool(name="vsb", bufs=1) as vsb_pool:
                    # All V transposes up front (own PSUM scope):
                    # v^T [d,s] -> V [s,d] per 128x128 tile
                    v_all_sb = vsb_pool.tile([P, HPC * KT, P], BF16)
                    with tc.tile_pool(name="ps_vtr", bufs=4,
                                      space="PSUM") as ps_vtr:
                        for h in range(HPC):
                            for kt in range(KT):
                                vt_ps = ps_vtr.tile([P, P], BF16, tag="vtr")
                                nc.tensor.transpose(vt_ps[:],
                                                    vT_sb[:, h, ts(kt, P)],
                                                    ident_sb[:])
                                nc.any.tensor_copy(
                                    v_all_sb[:, h * KT + kt, :], vt_ps[:])

                    with (
                        tc.tile_pool(name="exp", bufs=6) as exp_pool,
                        tc.tile_pool(name="rcp", bufs=2) as rcp_pool,
                        tc.tile_pool(name="ps_sc", bufs=3, space="PSUM") as ps_sc,
                        tc.tile_pool(name="ps_av", bufs=2, space="PSUM") as ps_av,
                        tc.tile_pool(name="ps_den", bufs=2, space="PSUM") as ps_den,
                    ):
                        for h in range(HPC):
                            v_sb = v_all_sb[:, h * KT:(h + 1) * KT, :]
                            qh = ropeT_sb[:, h, :]
                            kh = ropeT_sb[:, HPC + h, :]
                            for j in range(NSBLK):
                                nkt = 4 * j + 4
                                av_ps = ps_av.tile([P, SBLK], F32, tag="av")
                                den_ps = ps_den.tile([P, SBLK], F32, tag="den")
                                for i in range(nkt):
                                    sc_ps = ps_sc.tile([P, SBLK], F32, tag="sc")
                                    nc.tensor.matmul(sc_ps[:], kh[:, ts(i, P)],
                                                     qh[:, ts(j, SBLK)],
                                                     start=True, stop=True)
                                    exp_sb = exp_pool.tile([P, SBLK], BF16,
                                                           tag="exp")
                                    nc.scalar.activation(
                                        exp_sb[:], sc_ps[:],
                                        mybir.ActivationFunctionType.Exp,
                                        scale=SCALE)
                                    if i >= 4 * j:
                                        nc.vector.tensor_mul(
                                            out=exp_sb[:], in0=exp_sb[:],
                                            in1=masks_sb[:, i - 4 * j, :])
                                    # all-ones lhsT -> the k-sum lands in PSUM
                                    # replicated across all 128 partitions
                                    nc.tensor.matmul(
                                        den_ps[:], ones_sb[:], exp_sb[:],
                                        start=(i == 0), stop=(i == nkt - 1))
                                    nc.tensor.matmul(
                                        av_ps[:], v_sb[:, i, :], exp_sb[:],
                                        start=(i == 0), stop=(i == nkt - 1))
                                recip_sb = rcp_pool.tile([P, SBLK], F32,
                                                         tag="rcp")
                                nc.vector.reciprocal(recip_sb[:], den_ps[:])
                                nc.vector.tensor_mul(
                                    out=attnT_sb[:, h, ts(j, SBLK)],
                                    in0=av_ps[:], in1=recip_sb[:])

                            if h == 1:
                                nc.sync.dma_start(ccinA_t, attnT_sb[:, 0:2, :])
                                nc.gpsimd.collective_compute(
                                    "AllGather", mybir.AluOpType.bypass,
                                    replica_groups=[list(range(NCORES))],
                                    ins=[ccinA.ap()], outs=[ccoutA.ap()],
                                )
                            elif h == 3:
                                nc.sync.dma_start(ccinB_t, attnT_sb[:, 2:4, :])
                                nc.gpsimd.collective_compute(
                                    "AllGather", mybir.AluOpType.bypass,
                                    replica_groups=[list(range(NCORES))],
                                    ins=[ccinB.ap()], outs=[ccoutB.ap()],
                                )

                # ---------------- Phase 3: o_proj ------------------------
                with (
                    tc.tile_pool(name="at", bufs=6) as at_pool,
                    tc.tile_pool(name="osb", bufs=3) as osb_pool,
                    tc.tile_pool(name="ps_out", bufs=6, space="PSUM") as ps_out,
                ):
                    # f-tile (c, hh) in ccoutA/B block c -> global head 4c+hh
                    # (A: hh in {0,1}; B: hh in {2,3}).  The PE queue runs in
                    # order, so emit the AG_A-half of the first LEAD groups
                    # before any AG_B-half: that gives AllGather B a ~25us
                    # runway of PE work to hide behind.
                    NST = S // P
                    LEAD = 5
                    o_tiles = {}

                    def emit_a(st):
                        a_sbA = at_pool.tile([P, 16, P], BF16, tag="atA")
                        nc.sync.dma_start(a_sbA[:], ccoutA_t[:, :, ts(st, P)])
                        o_ps = ps_out.tile([P, ECOLS], F32, tag="out")
                        o_tiles[st] = o_ps
                        for idx in range(16):
                            g = 4 * (idx // 2) + (idx % 2)
                            nc.tensor.matmul(o_ps[:], a_sbA[:, idx, :],
                                             wot_sb[:, g, :],
                                             start=(idx == 0), stop=False)

                    def emit_b(st):
                        a_sbB = at_pool.tile([P, 16, P], BF16, tag="atB")
                        nc.sync.dma_start(a_sbB[:], ccoutB_t[:, :, ts(st, P)])
                        o_ps = o_tiles.pop(st)
                        for idx in range(16):
                            g = 4 * (idx // 2) + 2 + (idx % 2)
                            nc.tensor.matmul(o_ps[:], a_sbB[:, idx, :],
                                             wot_sb[:, g, :],
                                             start=False, stop=(idx == 15))
                        o_sb = osb_pool.tile([P, ECOLS], F32, tag="osb")
                        nc.any.tensor_copy(o_sb[:], o_ps[:])
                        nc.sync.dma_start(out_ext.ap()[ts(st, P), :], o_sb[:])

                    for st in range(LEAD):
                        emit_a(st)
                    for st in range(NST):
                        if st + LEAD < NST:
                            emit_a(st + LEAD)
                        emit_b(st)

    nc.compile()
    _NC_CACHE = nc
    return nc


def _prep_inputs(hidden_states, cos, sin, w_pack, w_o):
    hs = np.asarray(hidden_states, dtype=np.float32).reshape(S, E)
    xt = np.ascontiguousarray(hs.T).astype(BF16NP)
    cost = np.ascontiguousarray(np.asarray(cos, dtype=np.float32).T)
    sint = np.ascontiguousarray(np.asarray(sin, dtype=np.float32).T)
    w_pack = np.asarray(w_pack, dtype=np.float32)
    w_o = np.asarray(w_o, dtype=np.float32)

    # rotate-half as a matmul: rot = R @ q  (R[d'<64, d'+64] = -1; R[d'>=64, d'-64] = +1)
    R = np.zeros((D, D), dtype=np.float32)
    half = D // 2
    for dp in range(half):
        R[dp, dp + half] = -1.0
    for dp in range(half, D):
        R[dp, dp - half] = 1.0
    rt = np.ascontiguousarray(R.T).astype(BF16NP)

    masks = np.zeros((4, P, SBLK), dtype=np.float32)
    kk = np.arange(P)[:, None]
    qq = np.arange(SBLK)[None, :]
    for r in range(4):
        masks[r] = (P * r + kk <= qq).astype(np.float32)
    masks = masks.astype(BF16NP)

    ones = np.ones((P, P), dtype=BF16NP)
    ident = np.eye(P, dtype=np.float32).astype(BF16NP)

    in_maps = []
    hw = E // NCORES  # 512 head-rows per core in each of q/k/v
    for c in range(NCORES):
        rows = slice(c * hw, (c + 1) * hw)
        wqkv = np.concatenate(
            [w_pack[rows], w_pack[E:][rows], w_pack[2 * E:][rows]], axis=0)
        wt = np.ascontiguousarray(wqkv.T).astype(BF16NP)
        wot = np.ascontiguousarray(w_o[rows].T).astype(BF16NP)
        in_maps.append({
            "xt": xt, "wt": wt, "wot": wot,
            "cost": cost, "sint": sint,
            "rt": rt, "masks": masks, "ones": ones, "ident": ident,
        })
    return in_maps


def run(trace=False, trace_cores=None, **inputs):
    nc = build()
    in_maps = _prep_inputs(**inputs)
    res = run_bass_kernel_spmd(
        nc, in_maps, core_ids=list(range(NCORES)),
        trace=trace, trace_cores=trace_cores,
    )
    out = np.concatenate([res.results[c]["out"] for c in range(NCORES)], axis=1)
    return out.reshape(B, S, E).astype(np.float32), res


def kernel(**inputs) -> np.ndarray:
    out, _ = run(trace=False, **inputs)
    return out


# revision 22
# speedup vs baseline: 1.1092x; 1.0395x over previous
"""Baichuan attention layer (B=1, S=2048, E=4096, H=32, D=128) on 8 Trainium2
NeuronCores.

Sharding:
- QKV projection + RoPE + causal attention: tensor-parallel by head (4 heads
  per core). All per-head tensors live in transposed [feature, seq] layout so
  every matmul contracts over the partition dim with zero transposes:
    qkv^T[f, s]   = W @ X^T                (lhsT = W^T tiles, rhs = X^T tiles)
    scores^T[k,q] = K @ Q^T                (lhsT = K^T tile, rhs = Q^T block)
    att^T[d, q]   = V^T @ P^T              (lhsT = V tile,   rhs = exp tile)
  Softmax runs without max-subtraction (scores ~ N(0,1) after 1/sqrt(D), fp32
  exp is safe); the denominator is accumulated with an all-ones [128,128] lhsT
  matmul so the k-sum lands in PSUM already replicated across partitions.
- One AllGather of att^T [512, 2048] bf16 per core -> full att^T [4096, 2048].
- o_proj: column-parallel (each core computes its 512 output columns for the
  full sequence, using its slice of w_o). Host concatenates along E.

All matmuls in bf16 with fp32 PSUM accumulation.
"""

import importlib.util
import sys
import types

import numpy as np
import ml_dtypes

BF16NP = ml_dtypes.bfloat16

B, S, E = 1, 2048, 4096
H, D = 32, 128
NCORES = 8
HPC = H // NCORES          # heads per core = 4
P = 128                    # partitions
SBLK = 512                 # seq block (matmul free dim)
NSBLK = S // SBLK          # 4
ET = E // P                # 32 e-tiles
NF = 3 * HPC               # 12 f-tiles per core (q0..3, k4..7, v8..11)
KT = S // P                # 16 k-tiles
ECOLS = E // NCORES        # 512 output columns per core
SCALE = 1.0 / float(np.sqrt(D))


def _install_ntff_hook():
    """antenv.axon_hooks is absent in this image; recreate it from trn_boot's
    ctypes shim so run_bass_kernel_spmd(trace=True) can capture NTFF traces."""
    if "antenv.axon_hooks" in sys.modules:
        return
    try:
        spec = importlib.util.spec_from_file_location(
            "trn_boot", "/root/.axon_site/trn_agent_boot/trn_boot.py")
        tb = importlib.util.module_from_spec(spec)
        spec.loader.exec_module(tb)
        hook = tb._ntff_profile_via_ctypes("/opt/axon/libaxon_pjrt.so")
    except Exception:
        hook = None
    mod = types.ModuleType("antenv.axon_hooks")
    mod.get_axon_ntff_profile_hook = lambda: hook
    mod.set_axon_ntff_profile_hook = lambda h: None
    sys.modules["antenv.axon_hooks"] = mod


_install_ntff_hook()

import concourse.bass as bass  # noqa: E402
import concourse.mybir as mybir  # noqa: E402
import concourse.tile as tile  # noqa: E402
from concourse import bacc  # noqa: E402
from concourse.bass import ts  # noqa: E402
from concourse.bass_utils import run_bass_kernel_spmd  # noqa: E402


def _maybe_patch_ldw_opt():
    """Optional experiment: walrus runs with --enable-ldw-opt=false; flipping
    it lets codegen pipeline LDWEIGHTS with matmuls.  Gated on BASS_LDW_OPT=1
    until proven correct."""
    import os
    if os.environ.get("BASS_LDW_OPT") != "1":
        return
    from concourse import bass_utils as bu
    if getattr(bu, "_ldw_patched", False):
        return
    orig_run = bu.run_command

    def patched(cmd, *a, **kw):
        if isinstance(cmd, list):
            cmd = [c.replace("--enable-ldw-opt=false", "--enable-ldw-opt=true")
                   if isinstance(c, str) else c for c in cmd]
        return orig_run(cmd, *a, **kw)

    bu.run_command = patched
    bu._ldw_patched = True


_maybe_patch_ldw_opt()

BF16 = mybir.dt.bfloat16
F32 = mybir.dt.float32

_NC_CACHE = None


def build():
    global _NC_CACHE
    if _NC_CACHE is not None:
        return _NC_CACHE
    nc = bacc.Bacc("TRN2", target_bir_lowering=False, debug=False,
                   num_devices=NCORES)

    xt_ext = nc.dram_tensor("xt", [E, S], BF16, kind="ExternalInput")
    wt_ext = nc.dram_tensor("wt", [E, NF * P], BF16, kind="ExternalInput")
    wot_ext = nc.dram_tensor("wot", [E, ECOLS], BF16, kind="ExternalInput")
    cost_ext = nc.dram_tensor("cost", [D, S], F32, kind="ExternalInput")
    sint_ext = nc.dram_tensor("sint", [D, S], F32, kind="ExternalInput")
    rt_ext = nc.dram_tensor("rt", [D, D], BF16, kind="ExternalInput")
    masks_ext = nc.dram_tensor("masks", [4, P, SBLK], BF16, kind="ExternalInput")
    ones_ext = nc.dram_tensor("ones", [P, P], BF16, kind="ExternalInput")
    ident_ext = nc.dram_tensor("ident", [P, P], BF16, kind="ExternalInput")
    out_ext = nc.dram_tensor("out", [S, ECOLS], F32, kind="ExternalOutput")

    # Two AllGathers (heads 0-1, heads 2-3) so the first overlaps the second
    # half of attention.  ccoutA rank-r block = rows [256r, 256r+256) =
    # global heads {4r, 4r+1}; ccoutB likewise heads {4r+2, 4r+3}.
    ccinA = nc.dram_tensor("ccinA", [2 * P, S], BF16)
    ccinB = nc.dram_tensor("ccinB", [2 * P, S], BF16)
    ccoutA = nc.dram_tensor("ccoutA", [NCORES * 2 * P, S], BF16,
                            addr_space="Shared")
    ccoutB = nc.dram_tensor("ccoutB", [NCORES * 2 * P, S], BF16,
                            addr_space="Shared")

    xt_t = xt_ext.ap().rearrange("(eo p) s -> p eo s", p=P)
    wt_t = wt_ext.ap().rearrange("(eo p) f -> p eo f", p=P)
    wot_t = wot_ext.ap().rearrange("(fo p) e -> p fo e", p=P)
    masks_t = masks_ext.ap().rearrange("r p q -> p r q")
    ccinA_t = ccinA.ap().rearrange("(h p) s -> p h s", p=P)
    ccinB_t = ccinB.ap().rearrange("(h p) s -> p h s", p=P)
    # [p, (c,h), s] with f-tile index (c, h) -> global head 4c + h(+2 for B)
    ccoutA_t = ccoutA.ap().rearrange("(c h p) s -> p (c h) s", p=P, h=2)
    ccoutB_t = ccoutB.ap().rearrange("(c h p) s -> p (c h) s", p=P, h=2)

    with tile.TileContext(nc) as tc:
        with (
            tc.tile_pool(name="cst", bufs=1) as cst,
            tc.tile_pool(name="ropeT", bufs=1) as ropeT_pool,
            tc.tile_pool(name="vT", bufs=1) as vT_pool,
        ):
            rt_sb = cst.tile([D, D], BF16)
            nc.sync.dma_start(rt_sb[:], rt_ext.ap())
            ones_sb = cst.tile([P, P], BF16)
            nc.sync.dma_start(ones_sb[:], ones_ext.ap())
            ident_sb = cst.tile([P, P], BF16)
            nc.sync.dma_start(ident_sb[:], ident_ext.ap())
            masks_sb = cst.tile([P, 4, SBLK], BF16)
            nc.sync.dma_start(masks_sb[:], masks_t)

            # q^T and k^T after RoPE: [128, 8, 2048]; v^T raw: [128, 4, 2048]
            ropeT_sb = ropeT_pool.tile([P, 2 * HPC, S], BF16)
            vT_sb = vT_pool.tile([P, HPC, S], BF16)

            # ---------------- Phase 1: QKV projection + RoPE -------------
            with (
                tc.tile_pool(name="xt", bufs=2) as xt_pool,
                tc.tile_pool(name="wq", bufs=3) as w_pool,
                tc.tile_pool(name="cs", bufs=2) as cs_pool,
                tc.tile_pool(name="qkc", bufs=3) as qkc_pool,
                tc.tile_pool(name="rtmp", bufs=4) as rtmp_pool,
                tc.tile_pool(name="ps_qkv", bufs=3, space="PSUM") as ps_qkv,
                tc.tile_pool(name="ps_rot", bufs=2, space="PSUM") as ps_rot,
            ):
                for b in range(NSBLK):
                    sblk = ts(b, SBLK)
                    xt_sb = xt_pool.tile([P, ET, SBLK], BF16, tag="xt")
                    # chunked so the first matmuls start before the whole
                    # block has landed (and chunks spread across DMA queues)
                    for ch in range(4):
                        nc.sync.dma_start(xt_sb[:, ts(ch, ET // 4), :],
                                          xt_t[:, ts(ch, ET // 4), sblk])
                    cos_sb = cs_pool.tile([D, SBLK], F32, tag="cos")
                    nc.sync.dma_start(cos_sb[:], cost_ext.ap()[:, sblk])
                    sin_sb = cs_pool.tile([D, SBLK], F32, tag="sin")
                    nc.sync.dma_start(sin_sb[:], sint_ext.ap()[:, sblk])

                    for f in range(NF):
                        w_sb = w_pool.tile([P, ET, P], BF16, tag="w")
                        nc.sync.dma_start(w_sb[:], wt_t[:, :, ts(f, P)])
                        acc_ps = ps_qkv.tile([P, SBLK], F32, tag="qkv")
                        for e in range(ET):
                            nc.tensor.matmul(
                                acc_ps[:], w_sb[:, e, :], xt_sb[:, e, :],
                                start=(e == 0), stop=(e == ET - 1),
                            )
                        if f < 2 * HPC:
                            # q/k: RoPE.  bf16 copy for the rotate matmul
                            qk_sb = qkc_pool.tile([P, SBLK], BF16, tag="qkc")
                            nc.any.tensor_copy(qk_sb[:], acc_ps[:])
                            rot_ps = ps_rot.tile([P, SBLK], F32, tag="rot")
                            nc.tensor.matmul(rot_ps[:], rt_sb[:], qk_sb[:],
                                             start=True, stop=True)
                            t1 = rtmp_pool.tile([P, SBLK], F32, tag="t1")
                            nc.vector.tensor_mul(out=t1[:], in0=acc_ps[:],
                                                 in1=cos_sb[:])
                            t2 = rtmp_pool.tile([P, SBLK], F32, tag="t2")
                            nc.vector.tensor_mul(out=t2[:], in0=rot_ps[:],
                                                 in1=sin_sb[:])
                            nc.vector.tensor_add(
                                out=ropeT_sb[:, f, sblk], in0=t1[:], in1=t2[:])
                        else:
                            # v: plain copy to bf16
                            nc.any.tensor_copy(vT_sb[:, f - 2 * HPC, sblk],
                                               acc_ps[:])

            # ------------- Phase 2: attention per head + AllGather -------
            with (
                tc.tile_pool(name="wot", bufs=1) as wot_pool,
                tc.tile_pool(name="attnT", bufs=1) as attnT_pool,
            ):
                wot_sb = wot_pool.tile([P, ET, ECOLS], BF16)
                nc.sync.dma_start(wot_sb[:], wot_t)
                attnT_sb = attnT_pool.tile([P, HPC, S], BF16)

                with tc.tile_pool(name="vsb", bufs=1) as vsb_pool:
                    # All V transposes up front (own PSUM scope):
                    # v^T [d,s] -> V [s,d] per 128x128 tile
                    v_all_sb = vsb_pool.tile([P, HPC * KT, P], BF16)
                    with tc.tile_pool(name="ps_vtr", bufs=4,
                                      space="PSUM") as ps_vtr:
                        for h in range(HPC):
                            for kt in range(KT):
                                vt_ps = ps_vtr.tile([P, P], BF16, tag="vtr")
                                nc.tensor.transpose(vt_ps[:],
                                                    vT_sb[:, h, ts(kt, P)],
                                                    ident_sb[:])
                                nc.any.tensor_copy(
                                    v_all_sb[:, h * KT + kt, :], vt_ps[:])

                    with (
                        tc.tile_pool(name="exp", bufs=6) as exp_pool,
                        tc.tile_pool(name="rcp", bufs=2) as rcp_pool,
                        tc.tile_pool(name="ps_sc", bufs=3, space="PSUM") as ps_sc,
                        tc.tile_pool(name="ps_av", bufs=2, space="PSUM") as ps_av,
                        tc.tile_pool(name="ps_den", bufs=2, space="PSUM") as ps_den,
                    ):
                        for h in range(HPC):
                            v_sb = v_all_sb[:, h * KT:(h + 1) * KT, :]
                            qh = ropeT_sb[:, h, :]
                            kh = ropeT_sb[:, HPC + h, :]
                            for j in range(NSBLK):
                                nkt = 4 * j + 4
                                av_ps = ps_av.tile([P, SBLK], F32, tag="av")
                                den_ps = ps_den.tile([P, SBLK], F32, tag="den")
                                for i in range(nkt):
                                    sc_ps = ps_sc.tile([P, SBLK], F32, tag="sc")
                                    nc.tensor.matmul(sc_ps[:], kh[:, ts(i, P)],
                                                     qh[:, ts(j, SBLK)],
                                                     start=True, stop=True)
                                    exp_sb = exp_pool.tile([P, SBLK], BF16,
                                                           tag="exp")
                                    nc.scalar.activation(
                                        exp_sb[:], sc_ps[:],
                                        mybir.ActivationFunctionType.Exp,
                                        scale=SCALE)
                                    if i >= 4 * j:
                                        nc.vector.tensor_mul(
                                            out=exp_sb[:], in0=exp_sb[:],
                                            in1=masks_sb[:, i - 4 * j, :])
                                    # all-ones lhsT -> the k-sum lands in PSUM
                                    # replicated across all 128 partitions
                                    nc.tensor.matmul(
                                        den_ps[:], ones_sb[:], exp_sb[:],
                                        start=(i == 0), stop=(i == nkt - 1))
                                    nc.tensor.matmul(
                                        av_ps[:], v_sb[:, i, :], exp_sb[:],
                                        start=(i == 0), stop=(i == nkt - 1))
                                recip_sb = rcp_pool.tile([P, SBLK], F32,
                                                         tag="rcp")
                                nc.vector.reciprocal(recip_sb[:], den_ps[:])
                                nc.vector.tensor_mul(
                                    out=attnT_sb[:, h, ts(j, SBLK)],
                                    in0=av_ps[:], in1=recip_sb[:])

                            if h == 1:
                                nc.sync.dma_start(ccinA_t, attnT_sb[:, 0:2, :])
                                nc.gpsimd.collective_compute(
                                    "AllGather", mybir.AluOpType.bypass,
                                    replica_groups=[list(range(NCORES))],
                                    ins=[ccinA.ap()], outs=[ccoutA.ap()],
                                )
                            elif h == 3:
                                nc.sync.dma_start(ccinB_t, attnT_sb[:, 2:4, :])
                                nc.gpsimd.collective_compute(
                                    "AllGather", mybir.AluOpType.bypass,
                                    replica_groups=[list(range(NCORES))],
                                    ins=[ccinB.ap()], outs=[ccoutB.ap()],
                                )

                # ---------------- Phase 3: o_proj ------------------------
                with (
                    tc.tile_pool(name="at", bufs=6) as at_pool,
                    tc.tile_pool(name="osb", bufs=3) as osb_pool,
                    tc.tile_pool(name="part", bufs=1) as part_pool,
                    tc.tile_pool(name="ps_out", bufs=6, space="PSUM") as ps_out,
                ):
                    # f-tile (c, hh) in ccoutA/B block c -> global head 4c+hh
                    # (A: hh in {0,1}; B: hh in {2,3}).  Two full passes: the
                    # A pass (~65us of PE work, needs only AllGather A) runs
                    # while AllGather B is still in flight; the B pass adds
                    # its half on top of the SBUF partials.
                    NST = S // P
                    part_sb = part_pool.tile([P, NST, ECOLS], F32)
                    for st in range(NST):
                        a_sbA = at_pool.tile([P, 16, P], BF16, tag="atA")
                        nc.sync.dma_start(a_sbA[:], ccoutA_t[:, :, ts(st, P)])
                        o_ps = ps_out.tile([P, ECOLS], F32, tag="out")
                        for idx in range(16):
                            g = 4 * (idx // 2) + (idx % 2)
                            nc.tensor.matmul(o_ps[:], a_sbA[:, idx, :],
                                             wot_sb[:, g, :],
                                             start=(idx == 0), stop=(idx == 15))
                        nc.any.tensor_copy(part_sb[:, st, :], o_ps[:])
                    for st in range(NST):
                        a_sbB = at_pool.tile([P, 16, P], BF16, tag="atB")
                        nc.sync.dma_start(a_sbB[:], ccoutB_t[:, :, ts(st, P)])
                        o_ps = ps_out.tile([P, ECOLS], F32, tag="out")
                        for idx in range(16):
                            g = 4 * (idx // 2) + 2 + (idx % 2)
                            nc.tensor.matmul(o_ps[:], a_sbB[:, idx, :],
                                             wot_sb[:, g, :],
                                             start=(idx == 0), stop=(idx == 15))
                        o_sb = osb_pool.tile([P, ECOLS], F32, tag="osb")
                        nc.vector.tensor_add(out=o_sb[:], in0=o_ps[:],
                                             in1=part_sb[:, st, :])
                        nc.sync.dma_start(out_ext.ap()[ts(st, P), :], o_sb[:])

    nc.compile()
    _NC_CACHE = nc
    return nc


def _prep_inputs(hidden_states, cos, sin, w_pack, w_o):
    hs = np.asarray(hidden_states, dtype=np.float32).reshape(S, E)
    xt = np.ascontiguousarray(hs.T).astype(BF16NP)
    cost = np.ascontiguousarray(np.asarray(cos, dtype=np.float32).T)
    sint = np.ascontiguousarray(np.asarray(sin, dtype=np.float32).T)
    w_pack = np.asarray(w_pack, dtype=np.float32)
    w_o = np.asarray(w_o, dtype=np.float32)

    # rotate-half as a matmul: rot = R @ q  (R[d'<64, d'+64] = -1; R[d'>=64, d'-64] = +1)
    R = np.zeros((D, D), dtype=np.float32)
    half = D // 2
    for dp in range(half):
        R[dp, dp + half] = -1.0
    for dp in range(half, D):
        R[dp, dp - half] = 1.0
    rt = np.ascontiguousarray(R.T).astype(BF16NP)

    masks = np.zeros((4, P, SBLK), dtype=np.float32)
    kk = np.arange(P)[:, None]
    qq = np.arange(SBLK)[None, :]
    for r in range(4):
        masks[r] = (P * r + kk <= qq).astype(np.float32)
    masks = masks.astype(BF16NP)

    ones = np.ones((P, P), dtype=BF16NP)
    ident = np.eye(P, dtype=np.float32).astype(BF16NP)

    in_maps = []
    hw = E // NCORES  # 512 head-rows per core in each of q/k/v
    for c in range(NCORES):
        rows = slice(c * hw, (c + 1) * hw)
        wqkv = np.concatenate(
            [w_pack[rows], w_pack[E:][rows], w_pack[2 * E:][rows]], axis=0)
        wt = np.ascontiguousarray(wqkv.T).astype(BF16NP)
        wot = np.ascontiguousarray(w_o[rows].T).astype(BF16NP)
        in_maps.append({
            "xt": xt, "wt": wt, "wot": wot,
            "cost": cost, "sint": sint,
            "rt": rt, "masks": masks, "ones": ones, "ident": ident,
        })
    return in_maps


def run(trace=False, trace_cores=None, **inputs):
    nc = build()
    in_maps = _prep_inputs(**inputs)
    res = run_bass_kernel_spmd(
        nc, in_maps, core_ids=list(range(NCORES)),
        trace=trace, trace_cores=trace_cores,
    )
    out = np.concatenate([res.results[c]["out"] for c in range(NCORES)], axis=1)
    return out.reshape(B, S, E).astype(np.float32), res


def kernel(**inputs) -> np.ndarray:
    out, _ = run(trace=False, **inputs)
    return out
